# revision 24
# baseline (speedup 1.0000x reference)
"""Grok1-style MoE (T=2048, H=1024, E=8, I=2048, top-2) on 8 Trainium2 cores.

Strategy (expert-parallel, per the sharding hint):
  - Host: compute the tiny router (x @ gate_w, tanh softcap, top-2, softmax)
    and dispatch tokens by expert assignment (the "all-to-all dispatch" step:
    with full inputs on the host, dispatch = gather per expert), packing the
    per-core shards in a device-friendly tiled layout, cast to bf16.
  - Device (SPMD, 1 expert per core): grouped GEMM
        gT = wg_e^T x_e^T ; uT = wu_e^T x_e^T   (computed transposed, [I, M])
        act = gelu_tanh(gT) * uT                ([I, M], bf16)
        yT  = wd_e^T @ act                      ([H, M], tokens stay on the
                                                 moving dim -> no 128-token
                                                 padding anywhere)
  - Host: combine = scatter-add prob_e * y_e^T into [T, H] (the "all-to-all
    combine weighted by router probs").

All matmuls run in bf16 (1 col/cycle on the 128x128 PE, same rate as fp32r,
but half the HBM traffic and 4x faster weight loads via FWL), accumulating
in fp32 PSUM.
"""

import numpy as np
import ml_dtypes

import concourse.mybir as mybir
import concourse.tile as tile
from concourse import bacc
from concourse.bass_utils import run_bass_kernel_spmd

T, H, E, I_DIM, TOPK = 2048, 1024, 8, 2048, 2
SOFTCAP = 30.0
P = 128
N_CORES = 8
KH = H // P       # 8 contraction tiles (phase 1)
NI = I_DIM // P   # 16 i tiles
NHT = H // P      # 8 output h tiles (phase 2)

BF16 = ml_dtypes.bfloat16

_compiled = {}
LAST_RESULTS = None


def _m_chunks(M_PAD):
    """Split [0, M_PAD) into EQUAL chunks <= 512 (M_PAD is pre-rounded)."""
    n = max(1, -(-M_PAD // 512))
    cl = M_PAD // n
    assert cl * n == M_PAD and cl % 4 == 0
    return [(i * cl, cl) for i in range(n)]


def _build(M_PAD):
    f32 = mybir.dt.float32
    bf16 = mybir.dt.bfloat16
    chunks = _m_chunks(M_PAD)

    nc = bacc.Bacc("TRN2", target_bir_lowering=False, num_devices=N_CORES)
    # Host-packed layouts (all DMAs contiguous per partition):
    #   xt  [KH, P, M]        : xt[k, p, m] = x_e[m, k*P+p]
    #   wgu [NI, 2, P, KH*P]  : wgu[it, 0, p, k*P+i] = wg_e[k*P+p, it*P+i]
    #                           wgu[it, 1, ...] same for wu_e
    #   wdp [NHT, P, NI*P]    : wdp[ht, p, it*P+h] = wd_e[it*P+p, ht*P+h]
    NCH = len(chunks)
    CL = chunks[0][1]
    xt = nc.dram_tensor("xt", [NCH, P, KH, CL], bf16, kind="ExternalInput")
    wgu = nc.dram_tensor("wgu", [NI, P, KH * 2 * P], bf16, kind="ExternalInput")
    wdp = nc.dram_tensor("wdp", [NHT, P, NI * P], bf16, kind="ExternalInput")
    y = nc.dram_tensor("y", [NHT * P, M_PAD], f32, kind="ExternalOutput")

    with tile.TileContext(nc) as tc:
        with (
            tc.tile_pool(name="persist", bufs=1) as persist,
            tc.tile_pool(name="wtiles", bufs=6) as wtiles,
            tc.tile_pool(name="outs", bufs=2) as outs,
            tc.tile_pool(name="psum", bufs=2, space="PSUM") as psum,
        ):
            xt_sb = persist.tile([P, NCH, KH, CL], bf16)
            acts = persist.tile([P, NI, M_PAD], bf16)
            wd_sb = persist.tile([P, NHT, NI * P], bf16)
            zero = persist.tile([P, 512], bf16)

            def wgu_src(it):
                return wgu.ap()[it].rearrange("p (k g i) -> p k g i", g=2, i=P)

            # PE warm-up: bridge part of the initial DMA wait with dummy
            # matmuls.  Deliberately SHORT (1.9us): the real matmuls then
            # start on the cold 1.2 GHz clock and stay continuously busy
            # through the DMA supply ramp, which keeps the HAM activity
            # window filled (single cold->warm flip).  A longer block sized
            # to latch HAM before the data lands measured WORSE across runs:
            # first-data time jitters 11.9-13.9us, and on slow-ramp runs the
            # warm stream idles between arrivals and HAM re-throttles.
            # 33 N=128 warm MMs = ~3.5us of continuous PE activity: enough
            # to latch the HAM clock gate to 2.4 GHz *during* the block
            # (needs 3.4us sustained), ending right at the ~11.7us
            # first-data time the chunk-major feed now delivers.  The real
            # stream then starts warm with zero cold-clock penalty.  Safe
            # against slow-ramp draws: a worst-case ~1.4us idle after the
            # block is far below the ~3.4us MID window that re-throttles.
            # (This sizing only works with the fast feed; with the earlier
            # 12.5-14.3us first-data times it measurably lost.)
            nc.gpsimd.memset(zero[:], 0.0)
            warm_ps = psum.tile([P, 512], f32, tag="warm")
            for _ in range(33):
                nc.tensor.matmul(
                    warm_ps[:, :P], zero[:, :P], zero[:, :P], start=True, stop=True
                )

            # Startup feed, both HWDGE rings.  Keep the number of in-flight
            # dma_starts LOW here: only ~12 DMA semaphores rotate, and the
            # 13th dma_start blocks its issuing engine until an earlier
            # DMA's semaphore is drained (measured: a blocked xt load
            # stalled the PE 1.5us and reset the HAM warm-up window).  So:
            # xt in 4 k-pair loads, wgu0 in 2 halves, 6 head dma_starts
            # total.  Critical pieces (wgu0 k0-3, xt k0-1) go on the sync
            # ring: scalar's first descriptor is delayed ~1.3us by its
            # ACT_TABLE_LOAD.
            wgu_sbs = {}
            wgu_sbs[0] = wtiles.tile([P, KH, 2, P], bf16, tag="wgu", name="wgu0")

            # xt is fed by (m-chunk, k-half) piece: the first matmul chunk
            # only reads columns [0, chunks[0]), so its full feed is 1.06MB
            # (wgu0 + xt chunk 0) instead of 1.6MB, split across both rings.
            # wgu0 itself goes in four k-pair pieces interleaved with the xt
            # pieces, ordered to match the interleaved g/u k-loop's
            # consumption: the first matmul needs only 265KB (wgu0 k0-1 +
            # xt k0-3 chunk 0), and each later piece lands just ahead of
            # its first consumer, so the stream starts ~2us earlier and
            # rides the DMA ramp with sub-0.5us stall fragments (which
            # keeps the HAM warm-up window accumulating).
            q = KH // 4
            nc.sync.dma_start(wgu_sbs[0][:, :q], wgu_src(0)[:, :q])
            nc.scalar.dma_start(wgu_sbs[0][:, q : 2 * q], wgu_src(0)[:, q : 2 * q])
            for ci, (m0, ml) in enumerate(chunks):
                nc.sync.dma_start(
                    xt_sb[:, ci, : KH // 2], xt.ap()[ci][:, : KH // 2]
                )
                if ci == 0:
                    nc.sync.dma_start(
                        wgu_sbs[0][:, 2 * q : 3 * q], wgu_src(0)[:, 2 * q : 3 * q]
                    )
                nc.scalar.dma_start(
                    xt_sb[:, ci, KH // 2 :], xt.ap()[ci][:, KH // 2 :]
                )
                if ci == 0:
                    nc.scalar.dma_start(
                        wgu_sbs[0][:, 3 * q :], wgu_src(0)[:, 3 * q :]
                    )

            # Phase 1: gT/uT = wg^T xT / wu^T xT per i-tile; act = gelu(g)*u.
            # wd (consumed only in phase 2) streams during the back half.
            for it in range(NI):
                if it not in wgu_sbs:
                    wgu_sbs[it] = wtiles.tile(
                        [P, KH, 2, P], bf16, tag="wgu", name=f"wgu{it}"
                    )
                    # Odd tiles on sync: wgu1 is the earliest-needed streamed
                    # tile and sync's ring starts ~1.3us before scalar's.
                    eng = nc.sync if it % 2 == 1 else nc.scalar
                    eng.dma_start(wgu_sbs[it][:], wgu_src(it))
                wgu_sb = wgu_sbs.pop(it)

                for ci, (m0, ml) in enumerate(chunks):
                    g_ps = psum.tile([P, ml], f32, tag="g")
                    u_ps = psum.tile([P, ml], f32, tag="u")
                    # g/u interleaved per k: each arriving xt/wgu piece
                    # unlocks 2 MMs, so ramp-time stalls fragment into
                    # halves that stay under the ~0.5us threshold that
                    # resets the HAM warm-up window.
                    for k in range(KH):
                        nc.tensor.matmul(
                            g_ps[:],
                            wgu_sb[:, k, 0],
                            xt_sb[:, ci, k],
                            start=(k == 0),
                            stop=(k == KH - 1),
                        )
                        nc.tensor.matmul(
                            u_ps[:],
                            wgu_sb[:, k, 1],
                            xt_sb[:, ci, k],
                            start=(k == 0),
                            stop=(k == KH - 1),
                        )
                    nc.scalar.activation(
                        acts[:, it, m0 : m0 + ml], g_ps[:],
                        mybir.ActivationFunctionType.Gelu_apprx_tanh,
                    )
                    nc.vector.tensor_mul(
                        acts[:, it, m0 : m0 + ml], acts[:, it, m0 : m0 + ml], u_ps[:]
                    )
                if it >= NI - NHT:
                    ht = it - (NI - NHT)
                    eng = nc.sync if it % 2 == 0 else nc.scalar
                    eng.dma_start(wd_sb[:, ht], wdp.ap()[ht])

            # Phase 2: yT[h, m] = sum_i wd[i, h] * act[i, m]  (tokens moving)
            for ht in range(NHT):
                for mi, (m0, ml) in enumerate(chunks):
                    d_ps = psum.tile([P, ml], f32, tag="d")
                    for it in range(NI):
                        nc.tensor.matmul(
                            d_ps[:],
                            wd_sb[:, ht, it * P : (it + 1) * P],
                            acts[:, it, m0 : m0 + ml],
                            start=(it == 0),
                            stop=(it == NI - 1),
                        )
                    y_sb = outs.tile([P, ml], f32, tag="y")
                    nc.scalar.activation(
                        y_sb[:], d_ps[:],
                        mybir.ActivationFunctionType.Copy,
                    )
                    # Alternate rings so the last two y tiles drain in
                    # parallel instead of queueing on one ring at the end.
                    eng = nc.sync if (ht + mi) % 2 == 0 else nc.scalar
                    eng.dma_start(
                        y.ap()[ht * P : (ht + 1) * P, m0 : m0 + ml], y_sb[:]
                    )

    nc.compile()
    return nc


def kernel(hidden_states, gate_w, wg, wu, wd):
    global LAST_RESULTS
    x = np.ascontiguousarray(np.asarray(hidden_states, dtype=np.float32))
    gw = np.asarray(gate_w, dtype=np.float32)
    wg = np.asarray(wg, dtype=np.float32)
    wu = np.asarray(wu, dtype=np.float32)
    wd = np.asarray(wd, dtype=np.float32)

    # Router on host (part of the dispatch/sharding step).
    logits = np.tanh((x @ gw) / np.float32(SOFTCAP))
    top2 = np.argsort(-logits, axis=1, kind="stable")[:, :TOPK]  # [T, 2]
    v = np.take_along_axis(logits, top2, axis=1)                 # descending
    ex = np.exp(v - v[:, :1])
    pk = (ex / ex.sum(axis=1, keepdims=True)).astype(np.float32)  # [T, 2]

    token_ids, probs_e = [], []
    for e in range(E):
        mask = top2 == e
        rows = np.where(mask.any(axis=1))[0]
        kk = np.argmax(mask[rows], axis=1)
        token_ids.append(rows)
        probs_e.append(pk[rows, kk])

    n_max = max(256, max(len(r) for r in token_ids))
    n_ch = max(1, -(-n_max // 512))
    M_PAD = n_ch * (-(-n_max // (4 * n_ch)) * 4)

    nc = _compiled.get(M_PAD)
    if nc is None:
        nc = _build(M_PAD)
        _compiled[M_PAD] = nc

    x_bf = x.astype(BF16)
    wg_bf = wg.astype(BF16)
    wu_bf = wu.astype(BF16)
    wd_bf = wd.astype(BF16)

    in_maps = []
    for e in range(E):
        ids = token_ids[e]
        xe = np.zeros((M_PAD, H), BF16)
        xe[: len(ids)] = x_bf[ids]
        # [M_PAD, KH, P] -> [KH, P, M_PAD] -> chunk-major [NCH, P, KH, CL]
        # (each (chunk, k-half) startup piece is then one contiguous
        #  2KB-class run per partition instead of 536B fragments)
        xt_full = xe.reshape(M_PAD, KH, P).transpose(1, 2, 0)
        xt_e = np.ascontiguousarray(
            np.stack([xt_full[:, :, m0 : m0 + cl].transpose(1, 0, 2)
                      for (m0, cl) in _m_chunks(M_PAD)])
        )
        # [H, I] -> [NI, P, KH*P]
        def pack_w(w_e):
            w4 = w_e.reshape(KH, P, NI, P)
            return w4.transpose(2, 1, 0, 3).reshape(NI, P, KH * P)
        def pack_w4(w_e):
            return w_e.reshape(KH, P, NI, P).transpose(2, 1, 0, 3)
        wgu_e = np.ascontiguousarray(
            np.stack([pack_w4(wg_bf[e]), pack_w4(wu_bf[e])], axis=3).reshape(
                NI, P, KH * 2 * P
            )
        )
        # [I, H] -> [NHT, P, NI*P]
        wdp_e = np.ascontiguousarray(
            wd_bf[e].reshape(NI, P, NHT, P).transpose(2, 1, 0, 3).reshape(
                NHT, P, NI * P
            )
        )
        in_maps.append({"xt": xt_e, "wgu": wgu_e, "wdp": wdp_e})

    res = run_bass_kernel_spmd(nc, in_maps, core_ids=list(range(N_CORES)))
    LAST_RESULTS = res

    out = np.zeros((T, H), np.float32)
    for e in range(E):
        ids = token_ids[e]
        yt = res.results[e]["y"][:, : len(ids)]                  # [H, n]
        out[ids] += (yt * probs_e[e][None, :]).T
    return out
